# revision 3
# baseline (speedup 1.0000x reference)
"""Biquad lowpass filter (torchaudio lowpass_biquad, SR=24000, cutoff=8000,
Q=0.707) over wav [64, 480000], data-parallel across 8 TRN2 NeuronCores.

The biquad's poles have |z| = sqrt(a2) ~= 0.49, so the IIR is numerically a
17-tap causal FIR (tail energy beyond tap 17 ~ 8e-6, far under the 2e-2
gate). The rel-err budget also admits bfloat16 end-to-end I/O (~2.8e-3
measured), which halves HBM traffic vs f32: the host casts wav to bf16, the
device reads bf16, computes the FIR in bf16 on the TensorEngine with f32
PSUM accumulation, writes bf16, and the host upcasts the result.

Layout per core: 8 rows x 16 chunks = 128 partitions of 30000 samples. Time
is cut into 125-sample slices, PE-transposed so within-slice time sits on
partitions, then used as the stationary operand of bf16 matmuls against a
banded 17-tap coefficient matrix H [125, 141]. Three slices form a PSUM bank
group (391 f32 cols): a 16-wide tail matmul from the previous group's last
slice defines the group head, then per slice a 16-wide accumulate (H[:,:16])
plus a 125-wide start=True matmul (H[:,16:141]) stagger-cover the bank so
every column is start-written exactly once. Transposes run one group ahead
of the conv matmuls so the PSUM->SBUF slab copy (scalar engine) hides under
PE work. y is copied out of PSUM as bf16, alternating DVE/Pool engines.
Input DMA rides the sync HWDGE ring, output the scalar ring.
"""

import sys

sys.path.insert(0, "/opt/trn_rl_repo")

import numpy as np
import ml_dtypes

import concourse.mybir as mybir
import concourse.tile as tile
from concourse import bacc
from concourse.bass_utils import run_bass_kernel_spmd

f32 = mybir.dt.float32
bf16 = mybir.dt.bfloat16

# ---- problem constants ----------------------------------------------------
SR = 24000
CUTOFF = 8000.0
Q = 0.707

B_FULL, T = 64, 480000
N_CORES = 8
R = B_FULL // N_CORES          # rows per core
NCH = 16                       # chunks per row
P = R * NCH                    # 128 partitions (one chunk each)
L = T // NCH                   # 30000 samples per chunk

LS = 125                       # slice length (time-on-partitions tile)
D = 17                         # FIR taps kept
TAILW = 16                     # cross-slice tail width (D - 1)
SLG = 3                        # slices per PSUM bank group
GW = SLG * LS                  # 375 samples per group
BANKW = GW + TAILW             # 391 f32 cols per PSUM bank (<=512)
NG = L // GW                   # 80 groups per chunk
IOG = 8                        # groups per DMA transfer
IOW = IOG * GW                 # 3000 samples per transfer (6000 B bf16)
NIO = NG // IOG                # 10 transfers each way

assert NG * GW == L and NIO * IOG == NG


def _fir_taps():
    w0 = 2.0 * np.pi * CUTOFF / SR
    alpha = np.sin(w0) / (2.0 * Q)
    cos_w0 = np.cos(w0)
    b0 = (1.0 - cos_w0) / 2.0
    b1 = 1.0 - cos_w0
    b2 = b0
    a0 = 1.0 + alpha
    a1 = -2.0 * cos_w0
    a2 = 1.0 - alpha
    b0, b1, b2, a1, a2 = (np.float32(b0 / a0), np.float32(b1 / a0),
                          np.float32(b2 / a0), np.float32(a1 / a0),
                          np.float32(a2 / a0))
    # impulse response in float64 using the float32-rounded coefficients
    h = np.zeros(D, dtype=np.float64)
    x1 = x2 = y1 = y2 = 0.0
    for t in range(D):
        x = 1.0 if t == 0 else 0.0
        y = (float(b0) * x + float(b1) * x1 + float(b2) * x2
             - float(a1) * y1 - float(a2) * y2)
        h[t] = y
        x2, x1 = x1, x
        y2, y1 = y1, y
    return h


def _const_block():
    """[128, LS+TAILW+128] bf16: banded H | identity."""
    h = _fir_taps()
    H = np.zeros((128, LS + TAILW), dtype=np.float32)
    for k in range(LS):
        for d in range(D):
            n = k + d
            if n < LS + TAILW:
                H[k, n] = h[d]
    ident = np.eye(128, dtype=np.float32)
    blk = np.concatenate([H, ident], axis=1)
    return blk.astype(ml_dtypes.bfloat16)


def _build():
    CONST_np = _const_block()
    nc = bacc.Bacc("TRN2", target_bir_lowering=False)

    wav = nc.dram_tensor("wav", [R, T], bf16, kind="ExternalInput")
    out = nc.dram_tensor("out", [R, T], bf16, kind="ExternalOutput")
    const_d = nc.inline_tensor(CONST_np, name="constblk")

    wav_ch = wav[:, :].rearrange("r (c l) -> (r c) l", c=NCH)   # [128, 30000]
    out_ch = out[:, :].rearrange("r (c l) -> (r c) l", c=NCH)

    with tile.TileContext(nc) as tc:
        with (
            tc.tile_pool(name="const", bufs=1) as cpool,
            tc.tile_pool(name="io", bufs=3) as iopool,
            tc.tile_pool(name="work", bufs=4) as wpool,
            tc.tile_pool(name="psum", bufs=3, space="PSUM") as ppool,
        ):
            cblk = cpool.tile([128, LS + TAILW + 128], bf16)
            nc.sync.dma_start(cblk[:], const_d[:, :])
            hA = cblk[:LS, 0: TAILW]                  # accumulate head
            hB = cblk[:LS, TAILW: LS + TAILW]         # start=True body
            hT = cblk[:LS, LS: LS + TAILW]            # cross-group tail
            ident = cblk[:, LS + TAILW:]

            # initial carry: the LS samples preceding each chunk (zeros for
            # row-initial chunks), transposed into slice layout.
            c0 = cpool.tile([P, LS], bf16)
            nc.gpsimd.memset(c0[:], 0.0)
            for r in range(R):
                nc.gpsimd.dma_start(
                    c0[r * NCH + 1: r * NCH + NCH, :],
                    wav_ch[r * NCH: r * NCH + NCH - 1, L - LS: L],
                )
            pc0 = ppool.tile([LS, P], bf16, tag="pt")
            nc.tensor.transpose(pc0[:], c0[:], ident)
            c0T = cpool.tile([LS, P], bf16)
            nc.scalar.copy(c0T[:], pc0[:, :])

            xin = {}
            yout = {}
            slabs = {}

            def start_io(io):
                xin[io] = iopool.tile([P, IOW], bf16, tag="xin",
                                      name=f"xin{io}")
                nc.sync.dma_start(xin[io][:],
                                  wav_ch[:, io * IOW: (io + 1) * IOW])
                yout[io] = iopool.tile([P, IOW], bf16, tag="yout",
                                       name=f"yout{io}")

            start_io(0)

            # transposes run one group ahead of the conv matmuls
            for g in range(NG + 1):
                if g < NG:
                    io = g // IOG
                    off = (g % IOG) * GW
                    if g % IOG == 0 and io + 1 < NIO:
                        start_io(io + 1)       # prefetch next transfer
                    pt = ppool.tile([LS, SLG * P], bf16, tag="pt")
                    for j in range(SLG):
                        nc.tensor.transpose(
                            pt[:, j * P: (j + 1) * P],
                            xin[io][:, off + j * LS: off + (j + 1) * LS],
                            ident,
                        )
                    slab = wpool.tile([LS, SLG * P], bf16, tag="slab",
                                      name=f"slab{g}")
                    nc.scalar.copy(slab[:], pt[:])
                    slabs[g] = slab

                if g >= 1:
                    gg = g - 1
                    io = gg // IOG
                    off = (gg % IOG) * GW
                    carry = (c0T[:, :] if gg == 0
                             else slabs[gg - 1][:, (SLG - 1) * P: SLG * P])
                    py = ppool.tile([P, BANKW], f32, tag="py")
                    nc.tensor.matmul(
                        py[:, 0: TAILW], carry, hT,
                        start=True, stop=False, skip_group_check=True,
                    )
                    for j in range(SLG):
                        sl = slabs[gg][:, j * P: (j + 1) * P]
                        nc.tensor.matmul(
                            py[:, j * LS: j * LS + TAILW], sl, hA,
                            start=False, stop=False, skip_group_check=True,
                        )
                        nc.tensor.matmul(
                            py[:, j * LS + TAILW: (j + 1) * LS + TAILW],
                            sl, hB,
                            start=True, stop=(j == SLG - 1),
                            skip_group_check=True,
                        )
                    nc.vector.tensor_copy(yout[io][:, off: off + GW],
                                          py[:, 0: GW])
                    if gg % IOG == IOG - 1:
                        nc.scalar.dma_start(
                            out_ch[:, io * IOW: (io + 1) * IOW],
                            yout[io][:])
                    if gg - 2 in slabs:
                        del slabs[gg - 2]

    nc.finalize()
    return nc


_NC_CACHE = None


def _get_nc():
    global _NC_CACHE
    if _NC_CACHE is None:
        _NC_CACHE = _build()
    return _NC_CACHE


def _run(wav_full: np.ndarray, trace: bool = False):
    global _NC_CACHE
    wav_full = np.ascontiguousarray(wav_full, dtype=np.float32)
    wav16 = wav_full.astype(ml_dtypes.bfloat16)
    in_maps = [
        {"wav": wav16[i * R: (i + 1) * R]} for i in range(N_CORES)
    ]
    last_err = None
    for attempt in range(3):
        try:
            res = run_bass_kernel_spmd(
                _get_nc(), in_maps, core_ids=list(range(N_CORES)), trace=trace
            )
            out = np.concatenate(
                [np.asarray(res.results[i]["out"]) for i in range(N_CORES)],
                axis=0)
            return out.astype(np.float32), res
        except Exception as e:          # transient device errors recover on retry
            last_err = e
            _NC_CACHE = None
            try:
                import jax
                jax.clear_caches()
            except Exception:
                pass
            import time
            time.sleep(5 * (attempt + 1))
    raise last_err


def kernel(wav: np.ndarray) -> np.ndarray:
    out, _ = _run(np.asarray(wav))
    return out
